# revision 27
# baseline (speedup 1.0000x reference)
"""Trainium2 Bass kernel for a Mamba-style SSM single step.

Reference math (fp32):
    delta = softplus(x @ W_delta @ W_dt + b_dt)        [U, D]
    B = x @ W_B ; C = x @ W_C                          [U, N]
    abar = exp(delta[:,:,None] * A[None,:,:])          [U, D, N]
    h_new = abar * h + (delta*x)[:,:,None] * B[:,None,:]
    y = einsum('udn,un->ud', h_new, C) + D_vec * x

Two SPMD launches over 8 cores:
  1. proj: [t|B|C] = x @ [W_delta|W_B|W_C], column-sharded (48/core).
     Host gathers the 128x384 result and transposes t.
  2. main: tensor-parallel over d_inner (1280/core): delta = softplus
     (t @ W_dt + b), then the elementwise state update in a layout with
     partition dim = users (U=128), free = (d, n).
"""

import os
import numpy as np

U, D_IN, RANK, N = 128, 10240, 320, 32
NCORES = 8
DSH = D_IN // NCORES            # 1280 per-core d shard
DT = int(os.environ.get("MAMBA_DT", "128"))   # d-tile size
NT = DSH // DT                  # tiles per core
CH = D_IN // 128                # contraction chunks for projections
W2 = RANK + 2 * N               # 384
WSL = W2 // NCORES              # 48 proj columns per core

_cache = {}
CHS = DSH // 128                # 10 contraction chunks per core in proj


def _register_scan_op():
    """Register a custom DVE op: out = cumsum(in0 * in1) along the free
    stream (per partition). uops sha is computed at registration."""
    from concourse import dve_ops
    from concourse.dve_spec import Spec, Src0, Src1, scan, AluOp, lower, _has_src1
    from concourse.dve_uop import DveOpSpec

    if hasattr(dve_ops, "MAMBA_MSUM"):
        return dve_ops.MAMBA_MSUM

    def _ref(in0, in1, s0, s1, imm2):
        P = in0.shape[0]
        a = np.asarray(in0, np.float32).reshape(P, -1)
        b = np.asarray(in1, np.float32).reshape(P, -1)
        return np.cumsum(a * b, axis=1, dtype=np.float32)

    spec = Spec(body=scan(AluOp.ADD, Src0 * Src1), reference=_ref)
    op = dve_ops.DveOp("MAMBA_MSUM", spec, subdim=False, uops_sha={})
    dve_ops.OPS.append(op)
    dve_ops.CUSTOM_DVE_SPECS[op.name] = spec
    dve_ops._SUB_OPCODE_FOR_NAME[op.name] = (
        dve_ops._CUSTOM_DVE_ROW_BASE + len(dve_ops.OPS) - 1)
    for ver in ("v3", "v4"):
        ds = DveOpSpec(
            name=op.name,
            opcode=dve_ops.get_dve_sub_opcode(op.name),
            uops=lower(spec, ver=ver),
            rd1_en=_has_src1(spec),
        )
        op.uops_sha[ver] = ds.sha(ver)
    dve_ops.MAMBA_MSUM = op
    return op


def _build_proj():
    import concourse.mybir as mybir
    import concourse.tile as tile
    from concourse import bacc
    from contextlib import ExitStack

    fp32 = mybir.dt.float32
    nc = bacc.Bacc("TRN2", target_bir_lowering=False, debug=False,
                   enable_asserts=False, num_devices=NCORES)
    # contraction-split: each core contracts its own 1280-row slice of
    # x^T and W_all over ALL 384 output columns; host sums the partials.
    xt_d = nc.dram_tensor("xTs", [DSH, U], fp32, kind="ExternalInput").ap()
    w_d = nc.dram_tensor("wsl", [DSH, W2], fp32, kind="ExternalInput").ap()
    o_d = nc.dram_tensor("tbc_out", [128, W2], fp32, kind="ExternalOutput").ap()

    with tile.TileContext(nc) as tc, ExitStack() as ctx:
        wpool = ctx.enter_context(tc.tile_pool(name="w", bufs=1))
        ppool = ctx.enter_context(tc.tile_pool(name="ps", bufs=1, space="PSUM"))
        spool = ctx.enter_context(tc.tile_pool(name="s", bufs=1))
        ps = ppool.tile([128, W2], fp32, tag="acc")
        xt_c = wpool.tile([128, CHS, U], fp32, tag="xt")
        nc.sync.dma_start(xt_c[:], xt_d.rearrange("(k p) u -> p k u", p=128))
        w_c = wpool.tile([128, CHS, W2], fp32, tag="w")
        nc.sync.dma_start(w_c[:], w_d.rearrange("(k p) w -> p k w", p=128))
        for k in range(CHS):
            nc.tensor.matmul(ps[:], lhsT=xt_c[:, k, :], rhs=w_c[:, k, :],
                             start=(k == 0), stop=(k == CHS - 1))
        out_sb = spool.tile([128, W2], fp32, tag="out")
        nc.scalar.copy(out_sb[:], ps[:])
        nc.sync.dma_start(o_d, out_sb[:])
    nc.compile()
    return nc


def _build_main():
    import concourse.mybir as mybir
    import concourse.tile as tile
    from concourse import bacc
    from contextlib import ExitStack

    fp32 = mybir.dt.float32
    AF = mybir.ActivationFunctionType
    OP = mybir.AluOpType
    scan_op = _register_scan_op()

    nc = bacc.Bacc("TRN2", target_bir_lowering=False, debug=False,
                   enable_asserts=False, num_devices=NCORES)

    h_d = nc.dram_tensor("h_in", [U, DSH, N], fp32, kind="ExternalInput").ap()
    x_d = nc.dram_tensor("x_sh", [U, DSH], fp32, kind="ExternalInput").ap()
    tta_d = nc.dram_tensor("tTa", [RANK + 1, U], fp32, kind="ExternalInput").ap()
    bc_d = nc.dram_tensor("bc_in", [128, 2 * N], fp32, kind="ExternalInput").ap()
    wdt_d = nc.dram_tensor("wdt_aug", [RANK + 1, DSH], fp32, kind="ExternalInput").ap()
    a_d = nc.dram_tensor("a_sh", [DSH, N], fp32, kind="ExternalInput").ap()
    dbc_d = nc.dram_tensor("dbc", [U, DSH], fp32, kind="ExternalInput").ap()
    hnew_d = nc.dram_tensor("h_out", [U, DSH, N], fp32, kind="ExternalOutput").ap()
    y_d = nc.dram_tensor("y_out", [U, DSH], fp32, kind="ExternalOutput").ap()

    with tile.TileContext(nc) as tc, ExitStack() as ctx:
        const = ctx.enter_context(tc.tile_pool(name="const", bufs=1))
        ppool = ctx.enter_context(tc.tile_pool(name="ps", bufs=2, space="PSUM"))
        hpool = ctx.enter_context(tc.tile_pool(name="h", bufs=3))
        apool = ctx.enter_context(tc.tile_pool(name="abc", bufs=3))
        bpool = ctx.enter_context(tc.tile_pool(name="bx", bufs=2))

        # ---------------- Phase P: delta projection ----------------
        x_sb = const.tile([U, DSH], fp32, tag="x")
        nc.sync.dma_start(x_sb[:], x_d)
        dbc_sb = const.tile([U, DSH], fp32, tag="dbc")
        nc.sync.dma_start(dbc_sb[:], dbc_d)
        wdt0 = const.tile([128, DSH], fp32, tag="wdt0")
        nc.sync.dma_start(wdt0[:], wdt_d[0:128, :])
        wdt1 = const.tile([128, DSH], fp32, tag="wdt1")
        nc.sync.dma_start(wdt1[:], wdt_d[128:256, :])
        wdt2 = const.tile([65, DSH], fp32, tag="wdt2")
        nc.sync.dma_start(wdt2[:], wdt_d[256:321, :])
        tT0 = const.tile([128, U], fp32, tag="tT0")
        nc.sync.dma_start(tT0[:], tta_d[0:128, :])
        tT1 = const.tile([128, U], fp32, tag="tT1")
        nc.sync.dma_start(tT1[:], tta_d[128:256, :])
        tT2 = const.tile([65, U], fp32, tag="tT2")
        nc.sync.dma_start(tT2[:], tta_d[256:321, :])
        bc_sb = const.tile([128, 2 * N], fp32, tag="bc")
        nc.sync.dma_start(bc_sb[:], bc_d)

        # delta = softplus(tT.T @ W_dt_aug): per-tile Exp, one Ln pass
        delta_sb = const.tile([U, DSH], fp32, tag="delta")
        DDT = 320
        for i in range(DSH // DDT):
            sl = slice(i * DDT, (i + 1) * DDT)
            d_ps = ppool.tile([U, DDT], fp32, tag="dps")
            nc.tensor.matmul(d_ps[:], lhsT=tT0[:], rhs=wdt0[:, sl], start=True, stop=False)
            nc.tensor.matmul(d_ps[:], lhsT=tT1[:], rhs=wdt1[:, sl], start=False, stop=False)
            nc.tensor.matmul(d_ps[:], lhsT=tT2[:], rhs=wdt2[:, sl], start=False, stop=True)
            if os.environ.get("MAMBA_ONELN", "1") == "1":
                nc.scalar.activation(delta_sb[:, sl], d_ps[:], AF.Exp)
            else:
                nc.scalar.activation(d_ps[:], d_ps[:], AF.Exp)
                nc.scalar.activation(delta_sb[:, sl], d_ps[:], AF.Ln, bias=1.0)
        if os.environ.get("MAMBA_ONELN", "1") == "1":
            nc.scalar.activation(delta_sb[:], delta_sb[:], AF.Ln, bias=1.0)

        # dx = delta * x ; y init = D * x
        dx_sb = const.tile([U, DSH], fp32, tag="dx")
        nc.vector.tensor_tensor(dx_sb[:], delta_sb[:], x_sb[:], op=OP.mult)
        y_sb = const.tile([U, DSH], fp32, tag="y")
        nc.vector.tensor_tensor(y_sb[:], x_sb[:], dbc_sb[:], op=OP.mult)

        # ---------------- Phase E: state update ----------------
        B_view = bc_sb[:, 0:N].unsqueeze(1).broadcast_to([U, DT, N])
        C_view = bc_sb[:, N:2 * N].unsqueeze(1).broadcast_to([U, DT, N])
        for i in range(NT):
            sl = slice(i * DT, (i + 1) * DT)
            abc = apool.tile([U, DT, N], fp32, tag="abc")
            a_src = a_d[sl, :].unsqueeze(0).broadcast_to([U, DT, N])
            nc.scalar.dma_start(abc[:], a_src)
            ht = hpool.tile([U, DT, N], fp32, tag="ht")
            nc.scalar.dma_start(ht[:], h_d[:, sl, :])
            # tmp = delta (x) A   (in place over abc)
            dview = delta_sb[:, sl].unsqueeze(2).broadcast_to([U, DT, N])
            nc.vector.tensor_tensor(abc[:], dview, abc[:], op=OP.mult)
            # abar = exp(tmp)  (in place)
            nc.scalar.activation(abc[:], abc[:], AF.Exp)
            # ah = abar * h    (in place over ht)
            nc.vector.tensor_tensor(ht[:], abc[:], ht[:], op=OP.mult)
            # bx = dx (x) B
            dxview = dx_sb[:, sl].unsqueeze(2).broadcast_to([U, DT, N])
            bxt = bpool.tile([U, DT, N], fp32, tag="bxt")
            if os.environ.get("MAMBA_GPS_BX", "1") == "1":
                nc.gpsimd.tensor_tensor(bxt[:], dxview, B_view, op=OP.mult)
            else:
                nc.vector.tensor_tensor(bxt[:], dxview, B_view, op=OP.mult)
            # h_new = ah + bx
            nc.vector.tensor_tensor(ht[:], bxt[:], ht[:], op=OP.add)
            if os.environ.get("MAMBA_SCAN", "1") == "1":
                # prefix = cumsum(h_new * C) along the tile's free stream
                # (into abc); per-d sums are prefix[d,N-1] - prefix[d-1,N-1].
                nc.vector._custom_dve(scan_op, out=abc[:], in0=ht[:], in1=C_view)
                s_last = abc[:, :, N - 1]
                nc.vector.tensor_tensor(y_sb[:, sl], s_last, y_sb[:, sl], op=OP.add)
                nc.vector.tensor_tensor(
                    y_sb[:, i * DT + 1:(i + 1) * DT],
                    y_sb[:, i * DT + 1:(i + 1) * DT],
                    abc[:, 0:DT - 1, N - 1],
                    op=OP.subtract,
                )
            else:
                nc.vector.tensor_tensor(abc[:], ht[:], C_view, op=OP.mult)
                yp = bpool.tile([U, DT], fp32, tag="yp")
                nc.vector.tensor_reduce(yp[:], abc[:], axis=mybir.AxisListType.X, op=OP.add)
                nc.vector.tensor_tensor(y_sb[:, sl], yp[:], y_sb[:, sl], op=OP.add)
            nc.sync.dma_start(hnew_d[:, sl, :], ht[:])
        nc.sync.dma_start(y_d, y_sb[:])

    nc.compile()
    return nc


def _get_modules():
    if "proj" not in _cache:
        _cache["proj"] = _build_proj()
        _cache["main"] = _build_main()
    return _cache["proj"], _cache["main"]


def _run(nc, in_maps, trace=False):
    from concourse import bass_utils
    return bass_utils.run_bass_kernel_spmd(
        nc, in_maps, core_ids=list(range(NCORES)), trace=trace,
    )


def _prep(x, h, W_delta, W_dt, b_dt, W_B, W_C, A, D):
    x = np.ascontiguousarray(np.asarray(x, np.float32))
    h = np.asarray(h, np.float32)
    wall = np.ascontiguousarray(
        np.concatenate(
            [np.asarray(W_delta, np.float32), np.asarray(W_B, np.float32),
             np.asarray(W_C, np.float32)], axis=1)
    )
    xt = np.ascontiguousarray(x.T)
    wdt_aug = np.ascontiguousarray(
        np.concatenate(
            [np.asarray(W_dt, np.float32),
             np.asarray(b_dt, np.float32)[None, :]], axis=0)
    )
    A = np.asarray(A, np.float32)
    D = np.asarray(D, np.float32)
    proj_maps = []
    for k in range(NCORES):
        sl = slice(k * DSH, (k + 1) * DSH)
        proj_maps.append({
            "xTs": np.ascontiguousarray(xt[sl, :]),
            "wsl": np.ascontiguousarray(wall[sl, :]),
        })
    main_common = {"x": x, "h": h, "wdt_aug": wdt_aug, "A": A, "D": D}
    return proj_maps, main_common


def _main_maps(common, tbc):
    x, h, wdt_aug, A, D = (common["x"], common["h"], common["wdt_aug"],
                           common["A"], common["D"])
    t = tbc[:, 0:RANK]
    bc = np.ascontiguousarray(tbc[:, RANK:W2])
    tta = np.ascontiguousarray(
        np.concatenate([t.T, np.ones((1, U), np.float32)], axis=0))
    in_maps = []
    for k in range(NCORES):
        sl = slice(k * DSH, (k + 1) * DSH)
        in_maps.append({
            "h_in": np.ascontiguousarray(h[:, sl, :]),
            "x_sh": np.ascontiguousarray(x[:, sl]),
            "tTa": tta,
            "bc_in": bc,
            "wdt_aug": np.ascontiguousarray(wdt_aug[:, sl]),
            "a_sh": np.ascontiguousarray(A[sl, :]),
            "dbc": np.ascontiguousarray(
                np.broadcast_to(D[sl][None, :], (U, DSH))),
        })
    return in_maps


def _gather(results):
    y = np.concatenate([results[k]["y_out"] for k in range(NCORES)], axis=1)
    h_new = np.concatenate([results[k]["h_out"] for k in range(NCORES)], axis=1)
    return y.astype(np.float32), h_new.astype(np.float32)


def _run_all(inputs, trace=False):
    nc_proj, nc_main = _get_modules()
    proj_maps, common = _prep(**inputs)
    res1 = _run(nc_proj, proj_maps, trace=trace)
    tbc = np.sum([res1.results[k]["tbc_out"] for k in range(NCORES)],
                 axis=0, dtype=np.float64).astype(np.float32)
    res2 = _run(nc_main, _main_maps(common, tbc), trace=trace)
    return _gather(res2.results), res1, res2


def kernel(x, h, W_delta, W_dt, b_dt, W_B, W_C, A, D):
    (y, h_new), _, _ = _run_all(dict(
        x=x, h=h, W_delta=W_delta, W_dt=W_dt, b_dt=b_dt,
        W_B=W_B, W_C=W_C, A=A, D=D), trace=False)
    return y, h_new


def _install_ntff_hook():
    """Shim antenv.axon_hooks (absent in this image) and register the
    ctypes NTFF profile hook so trace=True yields exec_time_ns."""
    import sys
    import types
    if "antenv.axon_hooks" not in sys.modules:
        import antenv
        mod = types.ModuleType("antenv.axon_hooks")
        mod._hook = None

        def set_axon_ntff_profile_hook(h):
            mod._hook = h

        def get_axon_ntff_profile_hook():
            return mod._hook

        mod.set_axon_ntff_profile_hook = set_axon_ntff_profile_hook
        mod.get_axon_ntff_profile_hook = get_axon_ntff_profile_hook
        sys.modules["antenv.axon_hooks"] = mod
        antenv.axon_hooks = mod
    import antenv.axon_hooks as ah
    if ah._hook is None:
        from trn_agent_boot.trn_boot import _ntff_profile_via_ctypes
        hook = _ntff_profile_via_ctypes("/opt/axon/libaxon_pjrt.so")
        if hook is not None:
            ah.set_axon_ntff_profile_hook(hook)
    from concourse import bass_utils
    bass_utils.upload_artifacts = lambda tmpdir: f"local:{tmpdir}"


def kernel_traced(x, h, W_delta, W_dt, b_dt, W_B, W_C, A, D):
    """Like kernel() but with NTFF tracing; returns ((y, h_new), res1, res2)."""
    _install_ntff_hook()
    out, res1, res2 = _run_all(dict(
        x=x, h=h, W_delta=W_delta, W_dt=W_dt, b_dt=b_dt,
        W_B=W_B, W_C=W_C, A=A, D=D), trace=True)
    return out, res1, res2


# revision 28
# speedup vs baseline: 1.0625x; 1.0625x over previous
"""Trainium2 Bass kernel for a Mamba-style SSM single step.

Reference math (fp32):
    delta = softplus(x @ W_delta @ W_dt + b_dt)        [U, D]
    B = x @ W_B ; C = x @ W_C                          [U, N]
    abar = exp(delta[:,:,None] * A[None,:,:])          [U, D, N]
    h_new = abar * h + (delta*x)[:,:,None] * B[:,None,:]
    y = einsum('udn,un->ud', h_new, C) + D_vec * x

Two SPMD launches over 8 cores:
  1. proj: [t|B|C] = x @ [W_delta|W_B|W_C], column-sharded (48/core).
     Host gathers the 128x384 result and transposes t.
  2. main: tensor-parallel over d_inner (1280/core): delta = softplus
     (t @ W_dt + b), then the elementwise state update in a layout with
     partition dim = users (U=128), free = (d, n).
"""

import os
import numpy as np

U, D_IN, RANK, N = 128, 10240, 320, 32
NCORES = 8
DSH = D_IN // NCORES            # 1280 per-core d shard
DT = int(os.environ.get("MAMBA_DT", "128"))   # d-tile size
NT = DSH // DT                  # tiles per core
CH = D_IN // 128                # contraction chunks for projections
W2 = RANK + 2 * N               # 384
WSL = W2 // NCORES              # 48 proj columns per core

_cache = {}
CHS = DSH // 128                # 10 contraction chunks per core in proj


def _register_scan_op():
    """Register a custom DVE op: out = cumsum(in0 * in1) along the free
    stream (per partition). uops sha is computed at registration."""
    from concourse import dve_ops
    from concourse.dve_spec import Spec, Src0, Src1, scan, AluOp, lower, _has_src1
    from concourse.dve_uop import DveOpSpec

    if hasattr(dve_ops, "MAMBA_MSUM"):
        return dve_ops.MAMBA_MSUM

    def _ref(in0, in1, s0, s1, imm2):
        P = in0.shape[0]
        a = np.asarray(in0, np.float32).reshape(P, -1)
        b = np.asarray(in1, np.float32).reshape(P, -1)
        return np.cumsum(a * b, axis=1, dtype=np.float32)

    spec = Spec(body=scan(AluOp.ADD, Src0 * Src1), reference=_ref)
    op = dve_ops.DveOp("MAMBA_MSUM", spec, subdim=False, uops_sha={})
    dve_ops.OPS.append(op)
    dve_ops.CUSTOM_DVE_SPECS[op.name] = spec
    dve_ops._SUB_OPCODE_FOR_NAME[op.name] = (
        dve_ops._CUSTOM_DVE_ROW_BASE + len(dve_ops.OPS) - 1)
    for ver in ("v3", "v4"):
        ds = DveOpSpec(
            name=op.name,
            opcode=dve_ops.get_dve_sub_opcode(op.name),
            uops=lower(spec, ver=ver),
            rd1_en=_has_src1(spec),
        )
        op.uops_sha[ver] = ds.sha(ver)
    dve_ops.MAMBA_MSUM = op
    return op


def _build_proj():
    import concourse.mybir as mybir
    import concourse.tile as tile
    from concourse import bacc
    from contextlib import ExitStack

    fp32 = mybir.dt.float32
    nc = bacc.Bacc("TRN2", target_bir_lowering=False, debug=False,
                   enable_asserts=False, num_devices=NCORES)
    # contraction-split: each core contracts its own 1280-row slice of
    # x^T and W_all over ALL 384 output columns; host sums the partials.
    xt_d = nc.dram_tensor("xTs", [DSH, U], fp32, kind="ExternalInput").ap()
    w_d = nc.dram_tensor("wsl", [DSH, W2], fp32, kind="ExternalInput").ap()
    o_d = nc.dram_tensor("tbc_out", [128, W2], fp32, kind="ExternalOutput").ap()

    with tile.TileContext(nc) as tc, ExitStack() as ctx:
        wpool = ctx.enter_context(tc.tile_pool(name="w", bufs=1))
        ppool = ctx.enter_context(tc.tile_pool(name="ps", bufs=1, space="PSUM"))
        spool = ctx.enter_context(tc.tile_pool(name="s", bufs=1))
        ps = ppool.tile([128, W2], fp32, tag="acc")
        xt_c = wpool.tile([128, CHS, U], fp32, tag="xt")
        nc.sync.dma_start(xt_c[:], xt_d.rearrange("(k p) u -> p k u", p=128))
        w_c = wpool.tile([128, CHS, W2], fp32, tag="w")
        nc.sync.dma_start(w_c[:], w_d.rearrange("(k p) w -> p k w", p=128))
        for k in range(CHS):
            nc.tensor.matmul(ps[:], lhsT=xt_c[:, k, :], rhs=w_c[:, k, :],
                             start=(k == 0), stop=(k == CHS - 1))
        out_sb = spool.tile([128, W2], fp32, tag="out")
        nc.scalar.copy(out_sb[:], ps[:])
        nc.sync.dma_start(o_d, out_sb[:])
    nc.compile()
    return nc


def _build_main():
    import concourse.mybir as mybir
    import concourse.tile as tile
    from concourse import bacc
    from contextlib import ExitStack

    fp32 = mybir.dt.float32
    AF = mybir.ActivationFunctionType
    OP = mybir.AluOpType
    scan_op = _register_scan_op()

    nc = bacc.Bacc("TRN2", target_bir_lowering=False, debug=False,
                   enable_asserts=False, num_devices=NCORES)

    h_d = nc.dram_tensor("h_in", [U, DSH, N], fp32, kind="ExternalInput").ap()
    x_d = nc.dram_tensor("x_sh", [U, DSH], fp32, kind="ExternalInput").ap()
    tta_d = nc.dram_tensor("tTa", [RANK + 1, U], fp32, kind="ExternalInput").ap()
    bc_d = nc.dram_tensor("bc_in", [128, 2 * N], fp32, kind="ExternalInput").ap()
    wdt_d = nc.dram_tensor("wdt_aug", [RANK + 1, DSH], fp32, kind="ExternalInput").ap()
    a_d = nc.dram_tensor("a_sh", [DSH, N], fp32, kind="ExternalInput").ap()
    dbc_d = nc.dram_tensor("dbc", [U, DSH], fp32, kind="ExternalInput").ap()
    hnew_d = nc.dram_tensor("h_out", [U, DSH, N], fp32, kind="ExternalOutput").ap()
    y_d = nc.dram_tensor("y_out", [U, DSH], fp32, kind="ExternalOutput").ap()

    with tile.TileContext(nc) as tc, ExitStack() as ctx:
        const = ctx.enter_context(tc.tile_pool(name="const", bufs=1))
        ppool = ctx.enter_context(tc.tile_pool(name="ps", bufs=2, space="PSUM"))
        hpool = ctx.enter_context(tc.tile_pool(name="h", bufs=3))
        apool = ctx.enter_context(tc.tile_pool(name="abc", bufs=3))
        bpool = ctx.enter_context(tc.tile_pool(name="bx", bufs=2))

        # ---------------- Phase P: delta projection ----------------
        x_sb = const.tile([U, DSH], fp32, tag="x")
        nc.sync.dma_start(x_sb[:], x_d)
        dbc_sb = const.tile([U, DSH], fp32, tag="dbc")
        nc.sync.dma_start(dbc_sb[:], dbc_d)
        wdt0 = const.tile([128, DSH], fp32, tag="wdt0")
        nc.sync.dma_start(wdt0[:], wdt_d[0:128, :])
        wdt1 = const.tile([128, DSH], fp32, tag="wdt1")
        nc.sync.dma_start(wdt1[:], wdt_d[128:256, :])
        wdt2 = const.tile([65, DSH], fp32, tag="wdt2")
        nc.sync.dma_start(wdt2[:], wdt_d[256:321, :])
        tT0 = const.tile([128, U], fp32, tag="tT0")
        nc.sync.dma_start(tT0[:], tta_d[0:128, :])
        tT1 = const.tile([128, U], fp32, tag="tT1")
        nc.sync.dma_start(tT1[:], tta_d[128:256, :])
        tT2 = const.tile([65, U], fp32, tag="tT2")
        nc.sync.dma_start(tT2[:], tta_d[256:321, :])
        bc_sb = const.tile([128, 2 * N], fp32, tag="bc")
        nc.sync.dma_start(bc_sb[:], bc_d)

        # delta = softplus(tT.T @ W_dt_aug): per-tile Exp, one Ln pass
        delta_sb = const.tile([U, DSH], fp32, tag="delta")
        DDT = 320
        for i in range(DSH // DDT):
            sl = slice(i * DDT, (i + 1) * DDT)
            d_ps = ppool.tile([U, DDT], fp32, tag="dps")
            nc.tensor.matmul(d_ps[:], lhsT=tT0[:], rhs=wdt0[:, sl], start=True, stop=False)
            nc.tensor.matmul(d_ps[:], lhsT=tT1[:], rhs=wdt1[:, sl], start=False, stop=False)
            nc.tensor.matmul(d_ps[:], lhsT=tT2[:], rhs=wdt2[:, sl], start=False, stop=True)
            if os.environ.get("MAMBA_ONELN", "1") == "1":
                nc.scalar.activation(delta_sb[:, sl], d_ps[:], AF.Exp)
            else:
                nc.scalar.activation(d_ps[:], d_ps[:], AF.Exp)
                nc.scalar.activation(delta_sb[:, sl], d_ps[:], AF.Ln, bias=1.0)
        if os.environ.get("MAMBA_ONELN", "1") == "1":
            nc.scalar.activation(delta_sb[:], delta_sb[:], AF.Ln, bias=1.0)

        # dx = delta * x ; y init = D * x
        dx_sb = const.tile([U, DSH], fp32, tag="dx")
        nc.vector.tensor_tensor(dx_sb[:], delta_sb[:], x_sb[:], op=OP.mult)
        y_sb = const.tile([U, DSH], fp32, tag="y")
        nc.vector.tensor_tensor(y_sb[:], x_sb[:], dbc_sb[:], op=OP.mult)

        # ---------------- Phase E: state update ----------------
        B_view = bc_sb[:, 0:N].unsqueeze(1).broadcast_to([U, DT, N])
        C_view = bc_sb[:, N:2 * N].unsqueeze(1).broadcast_to([U, DT, N])
        for i in range(NT):
            sl = slice(i * DT, (i + 1) * DT)
            abc = apool.tile([U, DT, N], fp32, tag="abc")
            a_src = a_d[sl, :].unsqueeze(0).broadcast_to([U, DT, N])
            nc.sync.dma_start(abc[:], a_src)
            ht = hpool.tile([U, DT, N], fp32, tag="ht")
            nc.sync.dma_start(ht[:], h_d[:, sl, :])
            # tmp = delta (x) A   (in place over abc)
            dview = delta_sb[:, sl].unsqueeze(2).broadcast_to([U, DT, N])
            nc.vector.tensor_tensor(abc[:], dview, abc[:], op=OP.mult)
            # abar = exp(tmp)  (in place)
            nc.scalar.activation(abc[:], abc[:], AF.Exp)
            # ah = abar * h    (in place over ht)
            nc.vector.tensor_tensor(ht[:], abc[:], ht[:], op=OP.mult)
            # bx = dx (x) B
            dxview = dx_sb[:, sl].unsqueeze(2).broadcast_to([U, DT, N])
            bxt = bpool.tile([U, DT, N], fp32, tag="bxt")
            if os.environ.get("MAMBA_GPS_BX", "1") == "1":
                nc.gpsimd.tensor_tensor(bxt[:], dxview, B_view, op=OP.mult)
            else:
                nc.vector.tensor_tensor(bxt[:], dxview, B_view, op=OP.mult)
            # h_new = ah + bx
            nc.vector.tensor_tensor(ht[:], bxt[:], ht[:], op=OP.add)
            if os.environ.get("MAMBA_SCAN", "1") == "1":
                # prefix = cumsum(h_new * C) along the tile's free stream
                # (into abc); per-d sums are prefix[d,N-1] - prefix[d-1,N-1].
                nc.vector._custom_dve(scan_op, out=abc[:], in0=ht[:], in1=C_view)
                s_last = abc[:, :, N - 1]
                nc.vector.tensor_tensor(y_sb[:, sl], s_last, y_sb[:, sl], op=OP.add)
                nc.vector.tensor_tensor(
                    y_sb[:, i * DT + 1:(i + 1) * DT],
                    y_sb[:, i * DT + 1:(i + 1) * DT],
                    abc[:, 0:DT - 1, N - 1],
                    op=OP.subtract,
                )
            else:
                nc.vector.tensor_tensor(abc[:], ht[:], C_view, op=OP.mult)
                yp = bpool.tile([U, DT], fp32, tag="yp")
                nc.vector.tensor_reduce(yp[:], abc[:], axis=mybir.AxisListType.X, op=OP.add)
                nc.vector.tensor_tensor(y_sb[:, sl], yp[:], y_sb[:, sl], op=OP.add)
            nc.sync.dma_start(hnew_d[:, sl, :], ht[:])
        nc.sync.dma_start(y_d, y_sb[:])

    nc.compile()
    return nc


def _get_modules():
    if "proj" not in _cache:
        _cache["proj"] = _build_proj()
        _cache["main"] = _build_main()
    return _cache["proj"], _cache["main"]


def _run(nc, in_maps, trace=False):
    from concourse import bass_utils
    return bass_utils.run_bass_kernel_spmd(
        nc, in_maps, core_ids=list(range(NCORES)), trace=trace,
    )


def _prep(x, h, W_delta, W_dt, b_dt, W_B, W_C, A, D):
    x = np.ascontiguousarray(np.asarray(x, np.float32))
    h = np.asarray(h, np.float32)
    wall = np.ascontiguousarray(
        np.concatenate(
            [np.asarray(W_delta, np.float32), np.asarray(W_B, np.float32),
             np.asarray(W_C, np.float32)], axis=1)
    )
    xt = np.ascontiguousarray(x.T)
    wdt_aug = np.ascontiguousarray(
        np.concatenate(
            [np.asarray(W_dt, np.float32),
             np.asarray(b_dt, np.float32)[None, :]], axis=0)
    )
    A = np.asarray(A, np.float32)
    D = np.asarray(D, np.float32)
    proj_maps = []
    for k in range(NCORES):
        sl = slice(k * DSH, (k + 1) * DSH)
        proj_maps.append({
            "xTs": np.ascontiguousarray(xt[sl, :]),
            "wsl": np.ascontiguousarray(wall[sl, :]),
        })
    main_common = {"x": x, "h": h, "wdt_aug": wdt_aug, "A": A, "D": D}
    return proj_maps, main_common


def _main_maps(common, tbc):
    x, h, wdt_aug, A, D = (common["x"], common["h"], common["wdt_aug"],
                           common["A"], common["D"])
    t = tbc[:, 0:RANK]
    bc = np.ascontiguousarray(tbc[:, RANK:W2])
    tta = np.ascontiguousarray(
        np.concatenate([t.T, np.ones((1, U), np.float32)], axis=0))
    in_maps = []
    for k in range(NCORES):
        sl = slice(k * DSH, (k + 1) * DSH)
        in_maps.append({
            "h_in": np.ascontiguousarray(h[:, sl, :]),
            "x_sh": np.ascontiguousarray(x[:, sl]),
            "tTa": tta,
            "bc_in": bc,
            "wdt_aug": np.ascontiguousarray(wdt_aug[:, sl]),
            "a_sh": np.ascontiguousarray(A[sl, :]),
            "dbc": np.ascontiguousarray(
                np.broadcast_to(D[sl][None, :], (U, DSH))),
        })
    return in_maps


def _gather(results):
    y = np.concatenate([results[k]["y_out"] for k in range(NCORES)], axis=1)
    h_new = np.concatenate([results[k]["h_out"] for k in range(NCORES)], axis=1)
    return y.astype(np.float32), h_new.astype(np.float32)


def _run_all(inputs, trace=False):
    nc_proj, nc_main = _get_modules()
    proj_maps, common = _prep(**inputs)
    res1 = _run(nc_proj, proj_maps, trace=trace)
    tbc = np.sum([res1.results[k]["tbc_out"] for k in range(NCORES)],
                 axis=0, dtype=np.float64).astype(np.float32)
    res2 = _run(nc_main, _main_maps(common, tbc), trace=trace)
    return _gather(res2.results), res1, res2


def kernel(x, h, W_delta, W_dt, b_dt, W_B, W_C, A, D):
    (y, h_new), _, _ = _run_all(dict(
        x=x, h=h, W_delta=W_delta, W_dt=W_dt, b_dt=b_dt,
        W_B=W_B, W_C=W_C, A=A, D=D), trace=False)
    return y, h_new


def _install_ntff_hook():
    """Shim antenv.axon_hooks (absent in this image) and register the
    ctypes NTFF profile hook so trace=True yields exec_time_ns."""
    import sys
    import types
    if "antenv.axon_hooks" not in sys.modules:
        import antenv
        mod = types.ModuleType("antenv.axon_hooks")
        mod._hook = None

        def set_axon_ntff_profile_hook(h):
            mod._hook = h

        def get_axon_ntff_profile_hook():
            return mod._hook

        mod.set_axon_ntff_profile_hook = set_axon_ntff_profile_hook
        mod.get_axon_ntff_profile_hook = get_axon_ntff_profile_hook
        sys.modules["antenv.axon_hooks"] = mod
        antenv.axon_hooks = mod
    import antenv.axon_hooks as ah
    if ah._hook is None:
        from trn_agent_boot.trn_boot import _ntff_profile_via_ctypes
        hook = _ntff_profile_via_ctypes("/opt/axon/libaxon_pjrt.so")
        if hook is not None:
            ah.set_axon_ntff_profile_hook(hook)
    from concourse import bass_utils
    bass_utils.upload_artifacts = lambda tmpdir: f"local:{tmpdir}"


def kernel_traced(x, h, W_delta, W_dt, b_dt, W_B, W_C, A, D):
    """Like kernel() but with NTFF tracing; returns ((y, h_new), res1, res2)."""
    _install_ntff_hook()
    out, res1, res2 = _run_all(dict(
        x=x, h=h, W_delta=W_delta, W_dt=W_dt, b_dt=b_dt,
        W_B=W_B, W_C=W_C, A=A, D=D), trace=True)
    return out, res1, res2


# revision 29
# speedup vs baseline: 1.0686x; 1.0058x over previous
"""Trainium2 Bass kernel for a Mamba-style SSM single step.

Reference math (fp32):
    delta = softplus(x @ W_delta @ W_dt + b_dt)        [U, D]
    B = x @ W_B ; C = x @ W_C                          [U, N]
    abar = exp(delta[:,:,None] * A[None,:,:])          [U, D, N]
    h_new = abar * h + (delta*x)[:,:,None] * B[:,None,:]
    y = einsum('udn,un->ud', h_new, C) + D_vec * x

Two SPMD launches over 8 cores:
  1. proj: [t|B|C] = x @ [W_delta|W_B|W_C], column-sharded (48/core).
     Host gathers the 128x384 result and transposes t.
  2. main: tensor-parallel over d_inner (1280/core): delta = softplus
     (t @ W_dt + b), then the elementwise state update in a layout with
     partition dim = users (U=128), free = (d, n).
"""

import os
import numpy as np

U, D_IN, RANK, N = 128, 10240, 320, 32
NCORES = 8
DSH = D_IN // NCORES            # 1280 per-core d shard
DT = int(os.environ.get("MAMBA_DT", "128"))   # d-tile size
NT = DSH // DT                  # tiles per core
CH = D_IN // 128                # contraction chunks for projections
W2 = RANK + 2 * N               # 384
WSL = W2 // NCORES              # 48 proj columns per core

_cache = {}
CHS = DSH // 128                # 10 contraction chunks per core in proj


def _register_scan_op():
    """Register a custom DVE op: out = cumsum(in0 * in1) along the free
    stream (per partition). uops sha is computed at registration."""
    from concourse import dve_ops
    from concourse.dve_spec import Spec, Src0, Src1, scan, AluOp, lower, _has_src1
    from concourse.dve_uop import DveOpSpec

    if hasattr(dve_ops, "MAMBA_MSUM"):
        return dve_ops.MAMBA_MSUM

    def _ref(in0, in1, s0, s1, imm2):
        P = in0.shape[0]
        a = np.asarray(in0, np.float32).reshape(P, -1)
        b = np.asarray(in1, np.float32).reshape(P, -1)
        return np.cumsum(a * b, axis=1, dtype=np.float32)

    spec = Spec(body=scan(AluOp.ADD, Src0 * Src1), reference=_ref)
    op = dve_ops.DveOp("MAMBA_MSUM", spec, subdim=False, uops_sha={})
    dve_ops.OPS.append(op)
    dve_ops.CUSTOM_DVE_SPECS[op.name] = spec
    dve_ops._SUB_OPCODE_FOR_NAME[op.name] = (
        dve_ops._CUSTOM_DVE_ROW_BASE + len(dve_ops.OPS) - 1)
    for ver in ("v3", "v4"):
        ds = DveOpSpec(
            name=op.name,
            opcode=dve_ops.get_dve_sub_opcode(op.name),
            uops=lower(spec, ver=ver),
            rd1_en=_has_src1(spec),
        )
        op.uops_sha[ver] = ds.sha(ver)
    dve_ops.MAMBA_MSUM = op
    return op


def _build_proj():
    import concourse.mybir as mybir
    import concourse.tile as tile
    from concourse import bacc
    from contextlib import ExitStack

    fp32 = mybir.dt.float32
    nc = bacc.Bacc("TRN2", target_bir_lowering=False, debug=False,
                   enable_asserts=False, num_devices=NCORES)
    # contraction-split: each core contracts its own 1280-row slice of
    # x^T and W_all over ALL 384 output columns; host sums the partials.
    xt_d = nc.dram_tensor("xTs", [DSH, U], fp32, kind="ExternalInput").ap()
    w_d = nc.dram_tensor("wsl", [DSH, W2], fp32, kind="ExternalInput").ap()
    o_d = nc.dram_tensor("tbc_out", [128, W2], fp32, kind="ExternalOutput").ap()

    with tile.TileContext(nc) as tc, ExitStack() as ctx:
        wpool = ctx.enter_context(tc.tile_pool(name="w", bufs=1))
        ppool = ctx.enter_context(tc.tile_pool(name="ps", bufs=1, space="PSUM"))
        spool = ctx.enter_context(tc.tile_pool(name="s", bufs=1))
        ps = ppool.tile([128, W2], fp32, tag="acc")
        xt_c = wpool.tile([128, CHS, U], fp32, tag="xt")
        nc.sync.dma_start(xt_c[:], xt_d.rearrange("(k p) u -> p k u", p=128))
        w_c = wpool.tile([128, CHS, W2], fp32, tag="w")
        nc.sync.dma_start(w_c[:], w_d.rearrange("(k p) w -> p k w", p=128))
        for k in range(CHS):
            nc.tensor.matmul(ps[:], lhsT=xt_c[:, k, :], rhs=w_c[:, k, :],
                             start=(k == 0), stop=(k == CHS - 1))
        out_sb = spool.tile([128, W2], fp32, tag="out")
        nc.scalar.copy(out_sb[:], ps[:])
        nc.sync.dma_start(o_d, out_sb[:])
    nc.compile()
    return nc


def _build_main():
    import concourse.mybir as mybir
    import concourse.tile as tile
    from concourse import bacc
    from contextlib import ExitStack

    fp32 = mybir.dt.float32
    AF = mybir.ActivationFunctionType
    OP = mybir.AluOpType
    scan_op = _register_scan_op()

    nc = bacc.Bacc("TRN2", target_bir_lowering=False, debug=False,
                   enable_asserts=False, num_devices=NCORES)

    h_d = nc.dram_tensor("h_in", [U, DSH, N], fp32, kind="ExternalInput").ap()
    x_d = nc.dram_tensor("x_sh", [U, DSH], fp32, kind="ExternalInput").ap()
    tta_d = nc.dram_tensor("tTa", [RANK + 1, U], fp32, kind="ExternalInput").ap()
    bc_d = nc.dram_tensor("bc_in", [128, 2 * N], fp32, kind="ExternalInput").ap()
    wdt_d = nc.dram_tensor("wdt_aug", [RANK + 1, DSH], fp32, kind="ExternalInput").ap()
    a_d = nc.dram_tensor("a_sh", [DSH, N], fp32, kind="ExternalInput").ap()
    dbc_d = nc.dram_tensor("dbc", [U, DSH], fp32, kind="ExternalInput").ap()
    hnew_d = nc.dram_tensor("h_out", [U, DSH, N], fp32, kind="ExternalOutput").ap()
    y_d = nc.dram_tensor("y_out", [U, DSH], fp32, kind="ExternalOutput").ap()

    with tile.TileContext(nc) as tc, ExitStack() as ctx:
        const = ctx.enter_context(tc.tile_pool(name="const", bufs=1))
        ppool = ctx.enter_context(tc.tile_pool(name="ps", bufs=2, space="PSUM"))
        hpool = ctx.enter_context(tc.tile_pool(name="h", bufs=3))
        apool = ctx.enter_context(tc.tile_pool(name="abc", bufs=3))
        bpool = ctx.enter_context(tc.tile_pool(name="bx", bufs=2))

        # ---------------- Phase P: delta projection ----------------
        # tTa + W_dt first: they gate the delta matmuls (critical path)
        tT0 = const.tile([128, U], fp32, tag="tT0")
        nc.sync.dma_start(tT0[:], tta_d[0:128, :])
        tT1 = const.tile([128, U], fp32, tag="tT1")
        nc.sync.dma_start(tT1[:], tta_d[128:256, :])
        tT2 = const.tile([65, U], fp32, tag="tT2")
        nc.sync.dma_start(tT2[:], tta_d[256:321, :])
        wdt0 = const.tile([128, DSH], fp32, tag="wdt0")
        nc.sync.dma_start(wdt0[:], wdt_d[0:128, :])
        wdt1 = const.tile([128, DSH], fp32, tag="wdt1")
        nc.sync.dma_start(wdt1[:], wdt_d[128:256, :])
        wdt2 = const.tile([65, DSH], fp32, tag="wdt2")
        nc.sync.dma_start(wdt2[:], wdt_d[256:321, :])
        x_sb = const.tile([U, DSH], fp32, tag="x")
        nc.sync.dma_start(x_sb[:], x_d)
        bc_sb = const.tile([128, 2 * N], fp32, tag="bc")
        nc.sync.dma_start(bc_sb[:], bc_d)
        dbc_sb = const.tile([U, DSH], fp32, tag="dbc")
        nc.sync.dma_start(dbc_sb[:], dbc_d)

        # delta = softplus(tT.T @ W_dt_aug): per-tile Exp, one Ln pass
        delta_sb = const.tile([U, DSH], fp32, tag="delta")
        DDT = 320
        for i in range(DSH // DDT):
            sl = slice(i * DDT, (i + 1) * DDT)
            d_ps = ppool.tile([U, DDT], fp32, tag="dps")
            nc.tensor.matmul(d_ps[:], lhsT=tT0[:], rhs=wdt0[:, sl], start=True, stop=False)
            nc.tensor.matmul(d_ps[:], lhsT=tT1[:], rhs=wdt1[:, sl], start=False, stop=False)
            nc.tensor.matmul(d_ps[:], lhsT=tT2[:], rhs=wdt2[:, sl], start=False, stop=True)
            if os.environ.get("MAMBA_ONELN", "1") == "1":
                nc.scalar.activation(delta_sb[:, sl], d_ps[:], AF.Exp)
            else:
                nc.scalar.activation(d_ps[:], d_ps[:], AF.Exp)
                nc.scalar.activation(delta_sb[:, sl], d_ps[:], AF.Ln, bias=1.0)
        if os.environ.get("MAMBA_ONELN", "1") == "1":
            nc.scalar.activation(delta_sb[:], delta_sb[:], AF.Ln, bias=1.0)

        # dx = delta * x ; y init = D * x
        dx_sb = const.tile([U, DSH], fp32, tag="dx")
        nc.vector.tensor_tensor(dx_sb[:], delta_sb[:], x_sb[:], op=OP.mult)
        y_sb = const.tile([U, DSH], fp32, tag="y")

        # ---------------- Phase E: state update ----------------
        B_view = bc_sb[:, 0:N].unsqueeze(1).broadcast_to([U, DT, N])
        C_view = bc_sb[:, N:2 * N].unsqueeze(1).broadcast_to([U, DT, N])
        for i in range(NT):
            sl = slice(i * DT, (i + 1) * DT)
            abc = apool.tile([U, DT, N], fp32, tag="abc")
            a_src = a_d[sl, :].unsqueeze(0).broadcast_to([U, DT, N])
            nc.sync.dma_start(abc[:], a_src)
            ht = hpool.tile([U, DT, N], fp32, tag="ht")
            nc.sync.dma_start(ht[:], h_d[:, sl, :])
            # tmp = delta (x) A   (in place over abc)
            dview = delta_sb[:, sl].unsqueeze(2).broadcast_to([U, DT, N])
            nc.vector.tensor_tensor(abc[:], dview, abc[:], op=OP.mult)
            # abar = exp(tmp)  (in place)
            nc.scalar.activation(abc[:], abc[:], AF.Exp)
            # ah = abar * h    (in place over ht)
            nc.vector.tensor_tensor(ht[:], abc[:], ht[:], op=OP.mult)
            # bx = dx (x) B
            dxview = dx_sb[:, sl].unsqueeze(2).broadcast_to([U, DT, N])
            bxt = bpool.tile([U, DT, N], fp32, tag="bxt")
            nc.vector.tensor_tensor(bxt[:], dxview, B_view, op=OP.mult)
            # h_new = ah + bx
            nc.vector.tensor_tensor(ht[:], bxt[:], ht[:], op=OP.add)
            # y tile init = D * x
            nc.vector.tensor_tensor(y_sb[:, sl], x_sb[:, sl], dbc_sb[:, sl], op=OP.mult)
            if os.environ.get("MAMBA_SCAN", "1") == "1":
                # prefix = cumsum(h_new * C) along the tile's free stream
                # (into abc); per-d sums are prefix[d,N-1] - prefix[d-1,N-1].
                nc.vector._custom_dve(scan_op, out=abc[:], in0=ht[:], in1=C_view)
                s_last = abc[:, :, N - 1]
                nc.vector.tensor_tensor(y_sb[:, sl], s_last, y_sb[:, sl], op=OP.add)
                nc.vector.tensor_tensor(
                    y_sb[:, i * DT + 1:(i + 1) * DT],
                    y_sb[:, i * DT + 1:(i + 1) * DT],
                    abc[:, 0:DT - 1, N - 1],
                    op=OP.subtract,
                )
            else:
                nc.vector.tensor_tensor(abc[:], ht[:], C_view, op=OP.mult)
                yp = bpool.tile([U, DT], fp32, tag="yp")
                nc.vector.tensor_reduce(yp[:], abc[:], axis=mybir.AxisListType.X, op=OP.add)
                nc.vector.tensor_tensor(y_sb[:, sl], yp[:], y_sb[:, sl], op=OP.add)
            nc.sync.dma_start(hnew_d[:, sl, :], ht[:])
        nc.sync.dma_start(y_d, y_sb[:])

    nc.compile()
    return nc


def _get_modules():
    if "proj" not in _cache:
        _cache["proj"] = _build_proj()
        _cache["main"] = _build_main()
    return _cache["proj"], _cache["main"]


def _run(nc, in_maps, trace=False):
    from concourse import bass_utils
    return bass_utils.run_bass_kernel_spmd(
        nc, in_maps, core_ids=list(range(NCORES)), trace=trace,
    )


def _prep(x, h, W_delta, W_dt, b_dt, W_B, W_C, A, D):
    x = np.ascontiguousarray(np.asarray(x, np.float32))
    h = np.asarray(h, np.float32)
    wall = np.ascontiguousarray(
        np.concatenate(
            [np.asarray(W_delta, np.float32), np.asarray(W_B, np.float32),
             np.asarray(W_C, np.float32)], axis=1)
    )
    xt = np.ascontiguousarray(x.T)
    wdt_aug = np.ascontiguousarray(
        np.concatenate(
            [np.asarray(W_dt, np.float32),
             np.asarray(b_dt, np.float32)[None, :]], axis=0)
    )
    A = np.asarray(A, np.float32)
    D = np.asarray(D, np.float32)
    proj_maps = []
    for k in range(NCORES):
        sl = slice(k * DSH, (k + 1) * DSH)
        proj_maps.append({
            "xTs": np.ascontiguousarray(xt[sl, :]),
            "wsl": np.ascontiguousarray(wall[sl, :]),
        })
    main_common = {"x": x, "h": h, "wdt_aug": wdt_aug, "A": A, "D": D}
    return proj_maps, main_common


def _main_maps(common, tbc):
    x, h, wdt_aug, A, D = (common["x"], common["h"], common["wdt_aug"],
                           common["A"], common["D"])
    t = tbc[:, 0:RANK]
    bc = np.ascontiguousarray(tbc[:, RANK:W2])
    tta = np.ascontiguousarray(
        np.concatenate([t.T, np.ones((1, U), np.float32)], axis=0))
    in_maps = []
    for k in range(NCORES):
        sl = slice(k * DSH, (k + 1) * DSH)
        in_maps.append({
            "h_in": np.ascontiguousarray(h[:, sl, :]),
            "x_sh": np.ascontiguousarray(x[:, sl]),
            "tTa": tta,
            "bc_in": bc,
            "wdt_aug": np.ascontiguousarray(wdt_aug[:, sl]),
            "a_sh": np.ascontiguousarray(A[sl, :]),
            "dbc": np.ascontiguousarray(
                np.broadcast_to(D[sl][None, :], (U, DSH))),
        })
    return in_maps


def _gather(results):
    y = np.concatenate([results[k]["y_out"] for k in range(NCORES)], axis=1)
    h_new = np.concatenate([results[k]["h_out"] for k in range(NCORES)], axis=1)
    return y.astype(np.float32), h_new.astype(np.float32)


def _run_all(inputs, trace=False):
    nc_proj, nc_main = _get_modules()
    proj_maps, common = _prep(**inputs)
    res1 = _run(nc_proj, proj_maps, trace=trace)
    tbc = np.sum([res1.results[k]["tbc_out"] for k in range(NCORES)],
                 axis=0, dtype=np.float64).astype(np.float32)
    res2 = _run(nc_main, _main_maps(common, tbc), trace=trace)
    return _gather(res2.results), res1, res2


def kernel(x, h, W_delta, W_dt, b_dt, W_B, W_C, A, D):
    (y, h_new), _, _ = _run_all(dict(
        x=x, h=h, W_delta=W_delta, W_dt=W_dt, b_dt=b_dt,
        W_B=W_B, W_C=W_C, A=A, D=D), trace=False)
    return y, h_new


def _install_ntff_hook():
    """Shim antenv.axon_hooks (absent in this image) and register the
    ctypes NTFF profile hook so trace=True yields exec_time_ns."""
    import sys
    import types
    if "antenv.axon_hooks" not in sys.modules:
        import antenv
        mod = types.ModuleType("antenv.axon_hooks")
        mod._hook = None

        def set_axon_ntff_profile_hook(h):
            mod._hook = h

        def get_axon_ntff_profile_hook():
            return mod._hook

        mod.set_axon_ntff_profile_hook = set_axon_ntff_profile_hook
        mod.get_axon_ntff_profile_hook = get_axon_ntff_profile_hook
        sys.modules["antenv.axon_hooks"] = mod
        antenv.axon_hooks = mod
    import antenv.axon_hooks as ah
    if ah._hook is None:
        from trn_agent_boot.trn_boot import _ntff_profile_via_ctypes
        hook = _ntff_profile_via_ctypes("/opt/axon/libaxon_pjrt.so")
        if hook is not None:
            ah.set_axon_ntff_profile_hook(hook)
    from concourse import bass_utils
    bass_utils.upload_artifacts = lambda tmpdir: f"local:{tmpdir}"


def kernel_traced(x, h, W_delta, W_dt, b_dt, W_B, W_C, A, D):
    """Like kernel() but with NTFF tracing; returns ((y, h_new), res1, res2)."""
    _install_ntff_hook()
    out, res1, res2 = _run_all(dict(
        x=x, h=h, W_delta=W_delta, W_dt=W_dt, b_dt=b_dt,
        W_B=W_B, W_C=W_C, A=A, D=D), trace=True)
    return out, res1, res2
